# revision 7
# baseline (speedup 1.0000x reference)
"""Causal multi-head self-attention (RoPE) Trainium2 Bass kernel.

Problem: x:(4,2048,1024), Wq/Wk/Wv:(1024,1024), Wo:(1024,1024), bo:(1024,)
  q,k,v = split_heads(x@W*), rope(q), rope(k), causal softmax(q k^T/8) v, @Wo+bo

Sharding: head-parallel across 8 cores. Core c owns heads {2c, 2c+1} for all
4 batches: it computes q/k/v projections against the 128-column weight slice,
attention for its heads, and a partial output projection against the matching
128-row slice of Wo. Host sums the 8 partial (8192,1024) outputs and adds bo.

On-core layout (all "T" tensors are feature-major: partitions=feature rows,
free=tokens):
  Q^T/K^T (128 x 2048/batch): rows = [h0 d-evens(32), h0 d-odds(32), h1 ...]
    (NeoX-style d-permutation, folded into the host-permuted weight columns;
     valid because q and k get the same permutation and qk^T is d-invariant)
  RoPE: Q <- Q*cos + (P2@Q)*sin2, where P2 swaps the even/odd halves per head
    (PE matmul) and sin2 carries the sign; DVE reads the projection PSUM
    directly (no staging copy).
  S^T tiles (tj x ti) = K^T.T @ Q^T per head (K=64 contraction, the PE
    double-pumps K<=64 fp16 so these stream at ~2 cols/cycle).
  A = exp(0.125*S^T) (ACT, one call covers both heads; straddle tiles
    band-masked with -1e30 triangle beforehand on DVE).
  O~^T (65 x ti) accumulated = [V|1].T @ A over tj chunks; row 64 = softmax
    denominators (ones column trick). Normalize: DVE reciprocal of the denom
    row, GPSIMD partition-broadcast, DVE multiply -> O^T rows (no DRAM
    round-trip).
  y partial (128t x 1024) = O^T-chunk.T @ Wo-slice, DMA'd psum->DRAM (fp32).

Scheduling: the attention inner loop is software-pipelined per 128-row K/V
chunk j (QK leads by 2 steps, exp by 1, AV trails); a cost-paced filler queue
interleaves the next batch's projection work and deferred output-projection
tiles into the attention steps so the PE never idles (which would also drop
its DVFS p-state).
"""

import numpy as np

B, T, C = 4, 2048, 1024
H, D = 16, 64
N_CORES = 8
BT = B * T
SCALE = 0.125  # D**-0.5
NEG = -1.0e30

TRACE = False            # set True (e.g. from test.py) to capture an NTFF trace
LAST_RESULT = None       # BassKernelResults of the most recent run

_BUILT = None            # cached nc


# --------------------------------------------------------------------------
# workaround: this walrus build rejects >1 semaphore wait per instruction
def _split_sem_waits(nc, max_waits=1):
    import concourse.mybir as mybir

    n = 0
    for f in nc.m.functions:
        for bb in f.blocks:
            insts = bb.instructions
            idx = 0
            while idx < len(insts):
                i = insts[idx]
                si = getattr(i, "sync_info", None)
                if si is not None and si.on_wait and len(si.on_wait) > max_waits:
                    waits = list(si.on_wait)
                    extra, keep = waits[:-max_waits], waits[-max_waits:]
                    si.on_wait = keep
                    pos = idx
                    for j in range(0, len(extra), max_waits):
                        n += 1
                        nd = mybir.InstNoOp(name=f"I-waitsplit-{n}", ins=[], outs=[])
                        nd.engine = i.engine
                        nd.sync_info = mybir.SyncInfo(
                            on_wait=extra[j : j + max_waits], on_update=[]
                        )
                        insts.insert(pos, nd)
                        pos += 1
                    idx = pos
                idx += 1


def _install_ntff_hook():
    """The image's antenv lacks axon_hooks; synthesize it so trace=True works."""
    import sys
    import types

    if "antenv.axon_hooks" in sys.modules:
        return
    import antenv

    state = {"hook": None}
    mod = types.ModuleType("antenv.axon_hooks")
    mod.get_axon_ntff_profile_hook = lambda: state["hook"]
    mod.set_axon_ntff_profile_hook = lambda h: state.__setitem__("hook", h)
    sys.modules["antenv.axon_hooks"] = mod
    antenv.axon_hooks = mod
    try:
        from trn_agent_boot.trn_boot import _ntff_profile_via_ctypes

        state["hook"] = _ntff_profile_via_ctypes("/opt/axon/libaxon_pjrt.so")
    except Exception:
        state["hook"] = None


# --------------------------------------------------------------------------
def _build():
    import concourse.bass as bass
    import concourse.mybir as mybir
    from concourse.tile import TileContext

    F = mybir.dt.float32
    MD = mybir.dt.float16  # matmul operand dtype
    MULT = mybir.AluOpType.mult
    ADD = mybir.AluOpType.add
    SUB = mybir.AluOpType.subtract
    EXP = mybir.ActivationFunctionType.Exp

    nc = bass.Bass()

    xT = nc.dram_tensor("xT", (C, BT), MD, kind="ExternalInput")
    wq = nc.dram_tensor("wq", (C, 128), MD, kind="ExternalInput")
    wk = nc.dram_tensor("wk", (C, 128), MD, kind="ExternalInput")
    wv = nc.dram_tensor("wv", (C, 128), MD, kind="ExternalInput")
    wo = nc.dram_tensor("wo", (128, C), MD, kind="ExternalInput")
    cosd = nc.dram_tensor("cos", (128, T), MD, kind="ExternalInput")
    sind = nc.dram_tensor("sin2", (128, T), MD, kind="ExternalInput")
    p2d = nc.dram_tensor("p2", (128, 128), MD, kind="ExternalInput")
    bandd = nc.dram_tensor("band2x", (128, 256), F, kind="ExternalInput")
    idd = nc.dram_tensor("idf", (128, 128), F, kind="ExternalInput")
    y = nc.dram_tensor("y", (BT, C), MD, kind="ExternalOutput")
    scr = nc.dram_tensor("scr", (B * 8, 512), F, kind="Internal")

    with TileContext(nc) as tc:
        with (
            tc.tile_pool(name="const", bufs=1) as cst,
            tc.tile_pool(name="xt", bufs=3) as xtp,
            tc.tile_pool(name="qt", bufs=2) as qp,
            tc.tile_pool(name="kt", bufs=2) as kp,
            tc.tile_pool(name="vt", bufs=2) as vp,
            tc.tile_pool(name="ot", bufs=2) as op_,
            tc.tile_pool(name="vst", bufs=2) as vstp,
            tc.tile_pool(name="qs", bufs=2) as qsp,
            tc.tile_pool(name="at", bufs=4) as ap_,
            tc.tile_pool(name="rr", bufs=4) as rrp,
            tc.tile_pool(name="bc", bufs=4) as bcp,
            tc.tile_pool(name="ys", bufs=4) as ysp,
            tc.tile_pool(name="sps", bufs=2, space="PSUM") as sps,
            tc.tile_pool(name="stp", bufs=2, space="PSUM") as stp,
            tc.tile_pool(name="avp", bufs=2, space="PSUM") as avp,
        ):
            # ---- constants (emission order = DMA need order) ---------------
            wq_t = cst.tile([128, 8, 128], MD)
            nc.sync.dma_start(
                out=wq_t, in_=wq[:, :].rearrange("(a p) c -> p a c", p=128))
            wk_t = cst.tile([128, 8, 128], MD)
            nc.sync.dma_start(
                out=wk_t, in_=wk[:, :].rearrange("(a p) c -> p a c", p=128))
            wv_t = cst.tile([128, 8, 128], MD)
            nc.sync.dma_start(
                out=wv_t, in_=wv[:, :].rearrange("(a p) c -> p a c", p=128))
            cos_t = cst.tile([128, T], MD)
            nc.sync.dma_start(out=cos_t, in_=cosd[:, :])
            sin_t = cst.tile([128, T], MD)
            nc.sync.dma_start(out=sin_t, in_=sind[:, :])
            p2_t = cst.tile([128, 128], MD)
            nc.sync.dma_start(out=p2_t, in_=p2d[:, :])
            id_t = cst.tile([128, 128], F)
            nc.sync.dma_start(out=id_t, in_=idd[:, :])
            band_t = cst.tile([128, 256], F)  # [band | band] for head pairs
            nc.sync.dma_start(out=band_t, in_=bandd[:, :])
            wo_t = cst.tile([128, C], MD)
            nc.sync.dma_start(out=wo_t, in_=wo[:, :])
            band2 = band_t[:, :].rearrange("p (a c) -> p a c", a=2)

            QKV = {}   # b -> (Qb, Kb, Vb)
            xts = {}   # (b, nb) -> xt tile

            def alloc_batch(b):
                Qb = qp.tile([128, T], MD, name="Qb")
                Kb = kp.tile([128, T], MD, name="Kb")
                # per head 65 cols per token-tile: [d 0..63 | ones]
                Vb = vp.tile([128, 16, 130], MD, name="Vb")
                QKV[b] = (Qb, Kb, Vb)
                ones = Vb[:, :, :].rearrange("p t (a c) -> p t a c", a=2)
                nc.gpsimd.memset(ones[:, :, :, 64], 1.0)

            # ---- projection unit: 512 tokens of batch b --------------------
            def proj_items(b, nb):
                """Returns a list of (cost_ns, fn) items; fn emits instrs."""
                Qb, Kb, Vb = QKV[b]
                g0 = b * T + nb * 512
                cols = slice(nb * 512, (nb + 1) * 512)
                items = []

                def it_xt():
                    xt = xtp.tile([128, 8, 512], MD, name="xt")
                    xts[(b, nb)] = xt
                    src = xT[:, :].rearrange("(a p) c -> p a c", p=128)
                    nc.sync.dma_start(out=xt, in_=src[:, :, g0 : g0 + 512])
                items.append((50, it_xt))

                state = {}

                def mk_ps(W):
                    def fn():
                        xt = xts[(b, nb)]
                        ps = sps.tile([128, 512], F, tag="p", name="ps")
                        for k in range(8):
                            nc.tensor.matmul(
                                ps[:, :], lhsT=W[:, k, :], rhs=xt[:, k, :],
                                start=(k == 0), stop=(k == 7),
                            )
                        state["ps"] = ps
                    return fn

                def mk_rope_mul(dst):
                    def fn():
                        ps = state["ps"]
                        qs = qsp.tile([128, 512], MD, name="qs")
                        nc.vector.tensor_tensor(qs[:, :], ps[:, :],
                                                sin_t[:, cols], MULT)
                        nc.vector.tensor_tensor(dst[:, cols], ps[:, :],
                                                cos_t[:, cols], MULT)
                        state["qs"] = qs
                    return fn

                def mk_rope_rot(dst):
                    def fn():
                        qs = state.pop("qs")
                        state.pop("ps")
                        rot = sps.tile([128, 512], F, tag="p", name="rot")
                        nc.tensor.matmul(rot[:, :], lhsT=p2_t[:, :],
                                         rhs=qs[:, :], start=True, stop=True)
                        nc.vector.tensor_tensor(dst[:, cols], dst[:, cols],
                                                rot[:, :], SUB)
                    return fn

                items.append((1750, mk_ps(wq_t)))
                items.append((100, mk_rope_mul(Qb)))
                items.append((300, mk_rope_rot(Qb)))
                items.append((1750, mk_ps(wk_t)))
                items.append((100, mk_rope_mul(Kb)))
                items.append((300, mk_rope_rot(Kb)))
                items.append((1750, mk_ps(wv_t)))

                def it_vst():
                    ps = state.pop("ps")
                    vst = vstp.tile([128, 512], F, name="vst")
                    nc.scalar.copy(vst[:, :], ps[:, :])
                    state["vst"] = vst
                items.append((100, it_vst))

                def mk_tr(tl):
                    def fn():
                        vst = state["vst"]
                        tt = nb * 4 + tl
                        tp = sps.tile([128, 128], F, tag="p", name="tp")
                        nc.tensor.transpose(
                            tp[:, :], vst[:, tl * 128 : (tl + 1) * 128], id_t)
                        dst = Vb[:, tt, :].rearrange("p (a c) -> p a c", a=2)
                        nc.vector.tensor_copy(
                            dst[:, :, 0:64],
                            tp[:, :].rearrange("p (a c) -> p a c", a=2))
                    return fn
                for tl in range(4):
                    items.append((250, mk_tr(tl)))
                return items

            # ---- output projection tile (128 tokens) -----------------------
            def mk_ytile(b, Ob, tt):
                def fn():
                    r0 = b * T + tt * 128
                    lhs = Ob[:, tt * 128 : (tt + 1) * 128]
                    ysb = ysp.tile([128, 1024], MD, name="ysb")
                    for nh in (0, 1):
                        yps = sps.tile([128, 512], F, tag="p", name="yps")
                        nc.tensor.matmul(
                            yps[:, :], lhsT=lhs,
                            rhs=wo_t[:, nh * 512 : (nh + 1) * 512],
                            start=True, stop=True,
                        )
                        dst = ysb[:, nh * 512 : (nh + 1) * 512]
                        if nh == 0:
                            nc.vector.tensor_copy(dst, yps[:, :])
                        else:
                            nc.scalar.copy(dst, yps[:, :])
                    nc.sync.dma_start(out=y[r0 : r0 + 128, :], in_=ysb[:, :])
                return fn

            # ---- filler queue ---------------------------------------------
            fillq = []   # list of (tag, cost, fn)

            def enq(tag, items):
                for cost, fn in items:
                    fillq.append((tag, cost, fn))

            def drain(pred):
                """Emit every queued item whose tag satisfies pred."""
                rest = []
                for tag, cost, fn in fillq:
                    if pred(tag):
                        fn()
                    else:
                        rest.append((tag, cost, fn))
                fillq[:] = rest

            def pop_budget(budget):
                while fillq and budget > 0:
                    tag, cost, fn = fillq.pop(0)
                    fn()
                    budget -= cost
                return budget

            # ---- attention for batch b ------------------------------------
            def phase_d(b):
                Qb, Kb, Vb = QKV[b]
                Ob = op_.tile([128, T], MD, name="Ob")
                for i in range(4):
                    # block i must see its projections emitted already
                    drain(lambda t, b=b, i=i: t[0] == "p" and
                          (t[1], t[2]) <= (b, i))
                    nch = 4 * i + 4
                    avh = [avp.tile([128, 512], F, name="av") for _ in (0, 1)]
                    sts = {}
                    As = {}

                    def emit_qk(j, i=i, sts=sts):
                        delta = j * 128 - i * 512
                        nl = 512 - max(0, delta)
                        off = 512 - nl
                        st = stp.tile([128, 2, 512], F, name="st")
                        for h in (0, 1):
                            hs = slice(64 * h, 64 * h + 64)
                            nc.tensor.matmul(
                                st[:, h, 0:nl],
                                lhsT=Kb[hs, j * 128 : (j + 1) * 128],
                                rhs=Qb[hs, i * 512 + off : (i + 1) * 512],
                                start=True, stop=True,
                            )
                        if delta >= 0:  # straddles the diagonal: mask triangle
                            nc.vector.tensor_tensor(
                                st[:, :, 0:128], st[:, :, 0:128], band2, ADD)
                        sts[j] = (st, off, nl)

                    def emit_exp(j, sts=sts, As=As):
                        st, off, nl = sts.pop(j)
                        A = ap_.tile([128, 2, 512], MD, name="A")
                        nc.scalar.activation(
                            A[:, :, 0:nl], st[:, :, 0:nl], EXP, scale=SCALE)
                        As[j] = (A, off, nl)

                    def emit_av(j, nch=nch, As=As, avh=avh):
                        A, off, nl = As.pop(j)
                        for h in (0, 1):
                            nc.tensor.matmul(
                                avh[h][0:65, off:512],
                                lhsT=Vb[:, j, 65 * h : 65 * h + 65],
                                rhs=A[:, h, 0:nl],
                                start=(j == 0), stop=(j == nch - 1),
                                skip_group_check=True,
                            )

                    for s in range(nch + 2):
                        if s < nch:
                            emit_qk(s)
                        if 1 <= s <= nch:
                            emit_exp(s - 1)
                        if s >= 2:
                            emit_av(s - 2)
                        pop_budget(700)

                    # normalize: O = O~ / denom (denom in row 64 of av).
                    # DVE wide reciprocal, then a DRAM bounce for the
                    # partition broadcast (stride-0 partition reads are only
                    # legal from DRAM).
                    for h in (0, 1):
                        row = b * 8 + i * 2 + h
                        rrow = rrp.tile([1, 512], F, name="rrow")
                        nc.vector.reciprocal(rrow[:, :], avh[h][64:65, 0:512])
                        nc.sync.dma_start(out=scr[row : row + 1, :],
                                          in_=rrow[:, :])
                        bct = bcp.tile([64, 512], F, name="bct")
                        src = scr[row : row + 1, :]
                        bap = bass.AP(
                            tensor=src.tensor, offset=src.offset,
                            ap=[[0, 64]] + [list(p) for p in src.ap[1:]],
                        )
                        nc.sync.dma_start(out=bct[:, :], in_=bap)
                        nc.vector.tensor_tensor(
                            Ob[64 * h : 64 * h + 64, i * 512 : (i + 1) * 512],
                            avh[h][0:64, 0:512], bct[:, :], MULT)

                    # defer the 4 output tiles of this block via the queue
                    for tt in range(4 * i, 4 * i + 4):
                        fillq.append((("y", b, i), 700, mk_ytile(b, Ob, tt)))

            # ---- top-level schedule ---------------------------------------
            alloc_batch(0)
            for cost, fn in proj_items(0, 0):
                fn()                      # first unit inline: critical path
            for nb in range(1, 4):
                enq(("p", 0, nb), proj_items(0, nb))
            for b in range(B):
                if b + 1 < B:
                    alloc_batch(b + 1)
                    for nb in range(4):
                        enq(("p", b + 1, nb), proj_items(b + 1, nb))
                phase_d(b)
            drain(lambda t: True)

    _split_sem_waits(nc)
    return nc


# --------------------------------------------------------------------------
def _host_inputs(x, Wq, Wk, Wv, Wo):
    """Per-core input dicts (all shared arrays built once)."""
    BF = np.float16
    xT = np.ascontiguousarray(
        np.asarray(x, dtype=np.float32).reshape(BT, C).T).astype(BF)

    # NeoX d-permutation within each head: evens then odds
    dperm = np.concatenate([np.arange(0, D, 2), np.arange(1, D, 2)])

    inv_freq = (1.0 / (10000.0 ** (np.arange(0, D, 2) / D))).astype(np.float64)
    pos = np.arange(T, dtype=np.float64)
    ang = pos[None, :] * inv_freq[:, None]  # (32, T)
    cos32 = np.cos(ang).astype(np.float32)
    sin32 = np.sin(ang).astype(np.float32)
    cos_t = np.tile(np.vstack([cos32, cos32]), (2, 1))  # (128, T)
    sin_t = np.tile(np.vstack([-sin32, sin32]), (2, 1))  # (128, T), sign folded

    p2 = np.zeros((128, 128), dtype=np.float32)
    for hb in (0, 64):
        for i2 in range(32):
            p2[hb + i2, hb + 32 + i2] = 1.0
            p2[hb + 32 + i2, hb + i2] = 1.0

    band = np.where(
        np.arange(128)[None, :] < np.arange(128)[:, None], np.float32(NEG), 0.0
    ).astype(np.float32)
    band2x = np.concatenate([band, band], axis=1)  # (128, 256)
    idf = np.eye(128, dtype=np.float32)

    Wq = np.asarray(Wq, dtype=np.float32)
    Wk = np.asarray(Wk, dtype=np.float32)
    Wv = np.asarray(Wv, dtype=np.float32)
    Wo = np.asarray(Wo, dtype=np.float32)

    in_maps = []
    for c in range(N_CORES):
        sl = slice(128 * c, 128 * (c + 1))
        wq_c = Wq[:, sl].reshape(C, 2, D)[:, :, dperm].reshape(C, 128)
        wk_c = Wk[:, sl].reshape(C, 2, D)[:, :, dperm].reshape(C, 128)
        in_maps.append({
            "xT": xT,
            "wq": np.ascontiguousarray(wq_c).astype(BF),
            "wk": np.ascontiguousarray(wk_c).astype(BF),
            "wv": np.ascontiguousarray(Wv[:, sl]).astype(BF),
            "wo": np.ascontiguousarray(Wo[sl, :]).astype(BF),
            "cos": cos_t.astype(BF),
            "sin2": sin_t.astype(BF),
            "p2": p2.astype(BF),
            "band2x": band2x,
            "idf": idf,
        })
    return in_maps


def kernel(x, Wq, Wk, Wv, Wo, bo):
    global _BUILT, LAST_RESULT
    from concourse.bass_utils import run_bass_kernel_spmd

    if TRACE:
        _install_ntff_hook()

    if _BUILT is None:
        _BUILT = _build()
    nc = _BUILT

    in_maps = _host_inputs(x, Wq, Wk, Wv, Wo)

    last_err = None
    for attempt in range(3):
        try:
            res = run_bass_kernel_spmd(
                nc, in_maps, core_ids=list(range(N_CORES)), trace=TRACE
            )
            break
        except Exception as e:  # transient NRT device errors: retry
            last_err = e
            import time as _time

            _time.sleep(2.0)
    else:
        raise last_err
    LAST_RESULT = res

    acc = res.results[0]["y"].astype(np.float64)
    for c in range(1, N_CORES):
        acc = acc + res.results[c]["y"]
    out = acc.astype(np.float32) + np.asarray(bo, dtype=np.float32)[None, :]
    return out.reshape(B, T, C)
